# revision 1
# baseline (speedup 1.0000x reference)
"""Trainium2 Bass kernel for nn_Attention_5299989643989.

GQA attention forward (B=2, T=2048, C=1024, 16 q heads / 4 kv heads, D=64)
with value-embedding gating, rotary embedding, qk rms-norm, causal softmax.

Sharding: 8 cores = batch (2) x kv-head-group (4).  Each core computes its
4 q heads / 1 kv head end-to-end plus the Wo row-shard partial output; the
host sums the 4 partials per batch (the Wo all-reduce, done at unshard).

Per-core structure (fp32r matmuls throughout; ~178us/core in the TRN2
cost-model timeline, verified on hardware at rel err 2.6e-4):
  phase1a: per 128-token chunk one jammed projection matmul
           [q(256)|k(64)|v(64)|gate(1)|pad] accumulated over C in PSUM and
           copied to SBUF; per 4-chunk group: rope over all 20 head
           instances via 4D strided views, rms-norm rstd via bit-trick
           Newton rsqrt on DVE (no ACT Ln table loads), sigmoid gate via
           Tanh (ACT exp table stays resident), ve-gating on GPSIMD.
  phase1b: paired 2-head PE transposes into qT [128(h-pair rows),2,T] and
           row-duplicated kT2 [128,T] so odd heads run at partition base 64.
  phase2:  per (head, 512-query block): scoresT tiles [128 keys, <=512 live
           queries] = kT^T q (causal column narrowing), exp on ACT with the
           folded 1.2*1.2/sqrt(64) scale, triangular mask multiply on
           GPSIMD, then yT [65,512] += v_aug^T expT with a ones column
           producing softmax denominators for free; score tiles flow
           through a 4-slot PSUM pipeline with depth-2 software stagger.
  norm3:   denominator reciprocals (custom-DVE approx, base-0 full-tile
           aligned: the op miscomputes on HW with shifted PSUM inputs),
           PE outer-product broadcast, yT scaling, row-sharded Wo, DMA out.
  The whole thing is software-pipelined at emission: group bi+1's
  projections and DVE chain are emitted before phase2(bi); normalize/Wo of
  bi-1 fill the PE while bi's transposes wait on the DVE chain.
"""

import numpy as np

import concourse.bacc as bacc
import concourse.bass as bass
import concourse.tile as tile
from concourse import mybir
from concourse.masks import make_identity

f32 = mybir.dt.float32
f32r = mybir.dt.float32r
AF = mybir.ActivationFunctionType

B, T, C = 2, 2048, 1024
N_HEAD, N_KV_HEAD, D = 16, 4, 64
HQ = N_HEAD // N_KV_HEAD  # q heads per core = 4
P = 128
NT = T // P       # 16 token chunks
KC = C // P       # 8 contraction chunks
IB = 512          # query block
NBI = T // IB     # 4 query blocks
GRP = IB // P     # 4 token chunks per query block
SC = 1.2 * 1.2 / 8.0  # folded qk scale: rms 1.2 factors * 1/sqrt(64)
H32 = D // 2


def build_program():
    nc = bacc.Bacc("TRN2", target_bir_lowering=False, debug=False, num_devices=8)

    xT = nc.dram_tensor("xT", [C, T], f32, kind="ExternalInput")
    wr = nc.dram_tensor("wr", [C, 386], f32, kind="ExternalInput")
    cosd = nc.dram_tensor("cosd", [T, 32], f32, kind="ExternalInput")
    sind = nc.dram_tensor("sind", [T, 32], f32, kind="ExternalInput")
    ve3 = nc.dram_tensor("ve3", [T, D], f32, kind="ExternalInput")
    woT = nc.dram_tensor("woT", [2 * P, C], f32, kind="ExternalInput")
    tri = nc.dram_tensor("tri", [P, P], f32, kind="ExternalInput")
    out = nc.dram_tensor("out", [T, C], f32, kind="ExternalOutput")

    with tile.TileContext(nc) as tc:
        with (
            tc.tile_pool(name="consts", bufs=1) as consts,
            tc.tile_pool(name="resid", bufs=1) as resid,
            tc.tile_pool(name="xload", bufs=2) as xload,
            tc.tile_pool(name="rot", bufs=2) as rot,
            tc.tile_pool(name="small", bufs=4) as small,
            tc.tile_pool(name="exps", bufs=7) as exps,
            tc.tile_pool(name="outsb", bufs=2) as outsb,
            tc.tile_pool(name="psmm", bufs=2, space="PSUM") as psmm,
            tc.tile_pool(name="pssc", bufs=4, space="PSUM") as pssc,
            tc.tile_pool(name="psy", bufs=2, space="PSUM") as psy,
        ):
            # ---- resident loads ----
            wr_sb = consts.tile([P, KC, 386], f32r)
            for kc in range(KC):
                nc.sync.dma_start(
                    wr_sb[:, kc, :],
                    wr[kc * P : (kc + 1) * P, :].bitcast(f32r),
                )
            xt0 = xload.tile([P, KC, IB], f32r, name="xt0", tag="xt")
            for kc in range(KC):
                nc.scalar.dma_start(
                    xt0[:, kc, :],
                    xT[kc * P : (kc + 1) * P, 0:IB].bitcast(f32r),
                )
            cos_sb = consts.tile([P, NT, 32], f32)
            nc.sync.dma_start(cos_sb[:], cosd[:].rearrange("(n p) d -> p n d", p=P))
            sin_sb = consts.tile([P, NT, 32], f32)
            nc.sync.dma_start(sin_sb[:], sind[:].rearrange("(n p) d -> p n d", p=P))
            ve3_sb = consts.tile([P, NT, D], f32)
            nc.sync.dma_start(ve3_sb[:], ve3[:].rearrange("(n p) d -> p n d", p=P))
            tri_sb = consts.tile([P, P], f32)
            nc.sync.dma_start(tri_sb[:], tri[:])
            wo1_sb = consts.tile([P, C], f32r)
            nc.sync.dma_start(wo1_sb[:], woT[0:P, :].bitcast(f32r))
            wo2_sb = consts.tile([P, C], f32r)
            nc.sync.dma_start(wo2_sb[:], woT[P : 2 * P, :].bitcast(f32r))
            ident = consts.tile([P, P], f32)
            make_identity(nc, ident[:])
            ones64 = consts.tile([33, D], f32r)
            nc.sync.dma_start(
                ones64[:], tri[0:1, 0:D].to_broadcast((33, D)).bitcast(f32r)
            )
            eps_sb = consts.tile([P, 1], f32)
            nc.vector.memset(eps_sb[:], 1e-6)
            rsq_k = consts.tile([P, 1], mybir.dt.uint32)
            nc.vector.memset(rsq_k[:], 0x5F3759DF)
            zero_sb = consts.tile([P, 1], f32)
            nc.vector.memset(zero_sb[:], 0.0)

            # ---- residents written by the kernel ----
            qT = resid.tile([P, 2, T], f32r)   # [h0|h1] rows, [h2|h3] rows
            kT2 = resid.tile([P, T], f32r)     # kT duplicated in both row halves
            v_aug = resid.tile([P, NT, D + 1], f32r)  # v plus ones column
            nc.sync.dma_start(
                v_aug[:, :, D : D + 1],
                tri[0:1, 0:1].unsqueeze(1).to_broadcast((P, NT, 1)).bitcast(f32r),
            )
            yT1 = resid.tile([P, T], f32r)        # yT heads 0,1
            yT2 = resid.tile([P, T], f32r)        # yT heads 2,3
            rs1 = resid.tile([33, T], f32)   # denominators: h0 row0, h1 row32
            rs2 = resid.tile([33, T], f32)
            nc.gpsimd.memset(rs1[:], 1.0)
            nc.gpsimd.memset(rs2[:], 1.0)
            rc1 = resid.tile([33, T], f32)
            rc2 = resid.tile([33, T], f32)
            rr1 = resid.tile([33, T], f32r)   # f32r-rounded copies for the PE
            rr2 = resid.tile([33, T], f32r)

            def load_x(bi):
                xt = xload.tile([P, KC, IB], f32r, name=f"xt{bi}", tag="xt")
                nc.scalar.dma_start(
                    xt[:],
                    xT[:, bi * IB : (bi + 1) * IB]
                    .rearrange("(kc p) t -> p kc t", p=P)
                    .bitcast(f32r),
                )
                return xt

            def phase1(bi, xt, halves=1):
                qkr = rot.tile([P, GRP, 320], f32, tag="qkr", bufs=1)  # roped q|k
                pjg = rot.tile([P, GRP, 386], f32, tag="pjg")
                tgg = small.tile([P, GRP], f32, tag="tgg")
                for tl in range(GRP):
                    tc_ = bi * GRP + tl
                    pj = psmm.tile([P, 512], f32, tag="mm")
                    for kc in range(KC):
                        nc.tensor.matmul(
                            pj[:, 0:386],
                            xt[:, kc, tl * P : (tl + 1) * P],
                            wr_sb[:, kc, :],
                            start=(kc == 0),
                            stop=(kc == KC - 1),
                        )
                    nc.scalar.copy(pjg[:, tl, :], pj[:, 0:386])
                    nc.scalar.activation(
                        tgg[:, tl : tl + 1], pj[:, 384:385], AF.Tanh,
                        scale=0.5, bias=zero_sb[:],
                    )

                # rope + rms + rstd + normalize over the group, optionally in
                # two chunk-pair halves (shorter DVE chain before the
                # transposes, at the cost of ~16 extra small DVE ops).
                tmp = rot.tile([P, GRP, 160], f32, tag="tmp", bufs=1)
                sqg = rot.tile([P, GRP, 320], f32, tag="sqg", bufs=1)
                msg = small.tile([P, GRP * 5], f32, tag="msg")
                rstdg = small.tile([P, GRP * 5], f32, tag="rstdg")
                nwt = small.tile([P, GRP * 5], f32, tag="nwt")
                qkn = rot.tile([P, GRP, 320], f32, tag="qkn", bufs=1)
                gstep = GRP // halves
                for hf in range(halves):
                    g0, g1_ = gstep * hf, gstep * (hf + 1)
                    f0, f1 = 5 * gstep * hf, 5 * gstep * (hf + 1)
                    nf = f1 - f0
                    qv5 = pjg[:, g0:g1_, 0:320].rearrange(
                        "p g (h d) -> p g h d", d=D
                    )
                    ro5 = qkr[:, g0:g1_, :].rearrange("p g (h d) -> p g h d", d=D)
                    t5 = tmp[:, g0:g1_, :].rearrange("p g (h d) -> p g h d", d=H32)
                    cs = cos_sb[:, bi * GRP + g0 : bi * GRP + g1_, :]
                    sn = sin_sb[:, bi * GRP + g0 : bi * GRP + g1_, :]
                    cos5 = cs.unsqueeze(2).broadcast_to([P, g1_ - g0, 5, H32])
                    sin5 = sn.unsqueeze(2).broadcast_to([P, g1_ - g0, 5, H32])
                    q1 = qv5[:, :, :, 0:H32]
                    q2 = qv5[:, :, :, H32:D]
                    nc.vector.tensor_mul(ro5[:, :, :, 0:H32], q1, cos5)
                    nc.vector.tensor_mul(t5[:], q2, sin5)
                    nc.vector.tensor_add(
                        ro5[:, :, :, 0:H32], ro5[:, :, :, 0:H32], t5[:]
                    )
                    nc.vector.tensor_mul(ro5[:, :, :, H32:D], q2, cos5)
                    nc.vector.tensor_mul(t5[:], q1, sin5)
                    nc.vector.tensor_sub(
                        ro5[:, :, :, H32:D], ro5[:, :, :, H32:D], t5[:]
                    )

                    nc.scalar.square(sqg[:, g0:g1_, :], qkr[:, g0:g1_, :])
                    nc.vector.reduce_sum(
                        msg[:, f0:f1],
                        sqg[:, g0:g1_, :].rearrange("p g (h d) -> p (g h) d", d=D),
                        axis=mybir.AxisListType.X,
                    )
                    # m = mean + eps; rstd = m^-1/2 by bit-trick seed + two
                    # Newton iterations, entirely on DVE (no ACT Ln table).
                    nc.vector.tensor_scalar(
                        msg[:, f0:f1], msg[:, f0:f1], 1.0 / D, 1e-6,
                        op0=mybir.AluOpType.mult, op1=mybir.AluOpType.add,
                    )
                    rstdu = rstdg[:, f0:f1].bitcast(mybir.dt.uint32)
                    nc.vector.tensor_scalar(
                        rstdu, msg[:, f0:f1].bitcast(mybir.dt.uint32), 1, None,
                        op0=mybir.AluOpType.logical_shift_right,
                    )
                    nc.vector.tensor_sub(
                        rstdu,
                        rsq_k[:].broadcast_to([P, nf]).bitcast(mybir.dt.uint32),
                        rstdu,
                    )
                    for _ in range(2):
                        nc.vector.tensor_mul(
                            nwt[:, f0:f1], msg[:, f0:f1], rstdg[:, f0:f1]
                        )
                        nc.vector.tensor_mul(
                            nwt[:, f0:f1], nwt[:, f0:f1], rstdg[:, f0:f1]
                        )
                        nc.vector.tensor_scalar(
                            nwt[:, f0:f1], nwt[:, f0:f1], -0.5, 1.5,
                            op0=mybir.AluOpType.mult, op1=mybir.AluOpType.add,
                        )
                        nc.vector.tensor_mul(
                            rstdg[:, f0:f1], rstdg[:, f0:f1], nwt[:, f0:f1]
                        )
                    nc.vector.tensor_mul(
                        qkn[:, g0:g1_, :].rearrange("p g (h d) -> p (g h) d", d=D),
                        qkr[:, g0:g1_, :].rearrange("p g (h d) -> p (g h) d", d=D),
                        rstdg[:, f0:f1].unsqueeze(2).broadcast_to([P, nf, D]),
                    )
                # gate r = sigmoid(z) = 0.5 + 0.5*tanh(z/2); ve3 is 3*ve.
                # Emitted after the rms chain so the DVE reaches the chain
                # sooner; elementwise v work runs on the idle GPSIMD.
                rgg = small.tile([P, GRP], f32, tag="rgg")
                nc.vector.tensor_scalar(
                    rgg[:], tgg[:], 0.5, 0.5,
                    op0=mybir.AluOpType.mult, op1=mybir.AluOpType.add,
                )
                vtg = small.tile([P, GRP, D], f32, tag="vtg", bufs=1)
                nc.gpsimd.tensor_mul(
                    vtg[:],
                    ve3_sb[:, bi * GRP : (bi + 1) * GRP, :],
                    rgg[:].unsqueeze(2).broadcast_to([P, GRP, D]),
                )
                nc.gpsimd.tensor_add(
                    v_aug[:, bi * GRP : (bi + 1) * GRP, 0:D],
                    pjg[:, :, 320:384],
                    vtg[:],
                )
                phase1b.qkn = qkn

            def phase1b(bi):
                qkn = phase1b.qkn
                # transposes (two heads per [128,128] transpose)
                tpk = pssc.tile([D, 512], f32, tag="sc")
                for tl in range(GRP):
                    tc_ = bi * GRP + tl
                    tp = pssc.tile([P, 256], f32, tag="sc")
                    nc.tensor.transpose(
                        tp[:, 0:P], qkn[:, tl, 0:128], ident[:]
                    )
                    nc.tensor.transpose(
                        tp[:, P : 2 * P], qkn[:, tl, 128:256], ident[:]
                    )
                    nc.tensor.transpose(
                        tpk[:, tl * P : (tl + 1) * P], qkn[:, tl, 256:320], ident[:]
                    )
                    nc.scalar.copy(
                        qT[:, :, tc_ * P : (tc_ + 1) * P],
                        tp[:].rearrange("p (g t) -> p g t", g=2),
                    )
                nc.vector.tensor_copy(kT2[0:D, bi * IB : (bi + 1) * IB], tpk[:])
                nc.vector.tensor_copy(kT2[D:P, bi * IB : (bi + 1) * IB], tpk[:])

            def phase2(bi):
                for h in range(HQ):
                    yp = psy.tile([D + 1, 512], f32, tag="y")
                    njt = GRP * (bi + 1)
                    rr = D * (h % 2)
                    qTh = qT[rr : rr + D, h // 2, :]

                    def emit_score(jt):
                        dg = jt - GRP * bi
                        lo = max(dg, 0) * P
                        sp = pssc.tile([P, 512], f32, tag="sc", name="sp")
                        ex = exps.tile([P, 512], f32r, tag="ex", name="ex")
                        nc.tensor.matmul(
                            sp[:, lo:512],
                            kT2[rr : rr + D, jt * P : (jt + 1) * P],
                            qTh[:, bi * IB + lo : (bi + 1) * IB],
                            start=True,
                            stop=True,
                        )
                        nc.scalar.activation(
                            ex[:, lo:512], sp[:, lo:512], AF.Exp,
                            scale=SC, bias=zero_sb[:],
                        )
                        return ex, lo, dg

                    def emit_av(jt, ex, lo, dg):
                        if dg >= 0:
                            nc.gpsimd.tensor_mul(
                                ex[:, lo : lo + P], ex[:, lo : lo + P], tri_sb[:]
                            )
                        nc.tensor.matmul(
                            yp[:, lo:512],
                            v_aug[:, jt, :],
                            ex[:, lo:512],
                            start=(jt == 0),
                            stop=(jt == njt - 1),
                        )

                    pending = []
                    for jt in range(njt):
                        pending.append((jt, *emit_score(jt)))
                        if len(pending) > 2:
                            emit_av(*pending.pop(0))
                    for it in pending:
                        emit_av(*it)
                    rst = rs1 if h < 2 else rs2
                    rrow = 32 * (h % 2)
                    nc.vector.tensor_copy(
                        rst[rrow : rrow + 1, bi * IB : (bi + 1) * IB],
                        yp[D : D + 1, :],
                    )
                    ytp = yT1 if h < 2 else yT2
                    row = D * (h % 2)
                    nc.vector.tensor_copy(
                        ytp[row : row + D, bi * IB : (bi + 1) * IB], yp[0:D, :]
                    )

            def norm3(bi):
                for pair in range(2):
                    rst = rs1 if pair == 0 else rs2
                    rct = rc1 if pair == 0 else rc2
                    rrt = rr1 if pair == 0 else rr2
                    nc.vector.reciprocal_approx_fast(
                        rct[:, bi * IB : (bi + 1) * IB],
                        rst[:, bi * IB : (bi + 1) * IB],
                    )
                    nc.gpsimd.tensor_copy(
                        rrt[:, bi * IB : (bi + 1) * IB],
                        rct[:, bi * IB : (bi + 1) * IB],
                    )
                for h in range(HQ):
                    rrt = rr1 if h < 2 else rr2
                    row = 32 * (h % 2)
                    rbp = psy.tile([D, 512], f32, tag="y")
                    nc.tensor.matmul(
                        rbp[:],
                        ones64[row : row + 1, :],
                        rrt[row : row + 1, bi * IB : (bi + 1) * IB],
                        start=True,
                        stop=True,
                    )
                    ytp = yT1 if h < 2 else yT2
                    yrow = D * (h % 2)
                    nc.vector.tensor_mul(
                        ytp[yrow : yrow + D, bi * IB : (bi + 1) * IB],
                        ytp[yrow : yrow + D, bi * IB : (bi + 1) * IB],
                        rbp[:],
                    )
                for tl in range(GRP):
                    tc_ = bi * GRP + tl
                    for cb in range(2):
                        po = psmm.tile([P, 512], f32, tag="mm")
                        nc.tensor.matmul(
                            po[:],
                            yT1[:, tc_ * P : (tc_ + 1) * P],
                            wo1_sb[:, cb * 512 : (cb + 1) * 512],
                            start=True,
                            stop=False,
                        )
                        nc.tensor.matmul(
                            po[:],
                            yT2[:, tc_ * P : (tc_ + 1) * P],
                            wo2_sb[:, cb * 512 : (cb + 1) * 512],
                            start=False,
                            stop=True,
                        )
                        ob = outsb.tile([P, 512], f32, tag="ob")
                        nc.vector.tensor_copy(ob[:], po[:])
                        nc.sync.dma_start(
                            out[tc_ * P : (tc_ + 1) * P, cb * 512 : (cb + 1) * 512],
                            ob[:],
                        )

            # group-level software pipeline: next group's projections are
            # emitted before the previous group's normalize/Wo so the PE has
            # ready work while the denominator chain resolves.
            xts = {0: xt0}
            phase1(0, xts[0])
            phase1b(0)
            xts[1] = load_x(1)
            phase1(1, xts[1], halves=2)
            xts[2] = load_x(2)
            phase2(0)
            for bi in range(1, NBI):
                phase1b(bi)
                if bi + 1 < NBI:
                    phase1(bi + 1, xts[bi + 1])
                    if bi + 2 < NBI:
                        xts[bi + 2] = load_x(bi + 2)
                norm3(bi - 1)
                phase2(bi)
            norm3(NBI - 1)
    nc.compile()
    return nc


def make_core_inputs(x, ve, cos, sin, Wq, Wk, Wv, Wo, Wg):
    """Slice full inputs into the 8 per-core input maps (b-major, then group)."""
    cosf = np.ascontiguousarray(cos[0, :, 0, :], dtype=np.float32)  # [T, 32]
    sinf = np.ascontiguousarray(sin[0, :, 0, :], dtype=np.float32)
    tri = (np.arange(P)[:, None] <= np.arange(P)[None, :]).astype(np.float32)
    in_maps = []
    for c in range(8):
        b, g = c // N_KV_HEAD, c % N_KV_HEAD
        xTc = np.ascontiguousarray(x[b].T, dtype=np.float32)  # [C, T]
        wq = Wq[g * 256 : (g + 1) * 256, :]           # [256, C]
        wk = Wk[g * D : (g + 1) * D, :]               # [64, C]
        wv = Wv[g * D : (g + 1) * D, :]
        wg_col = np.zeros((C, 1), np.float32)
        wg_col[:12, 0] = Wg[g]
        wrc = np.concatenate(
            [wq.T, wk.T, wv.T, wg_col, np.zeros((C, 1), np.float32)], axis=1
        ).astype(np.float32)                          # [C, 386] (even pad for f32r)
        ve3 = np.ascontiguousarray(
            3.0 * ve[b, :, g * D : (g + 1) * D], dtype=np.float32
        )                                             # [T, 64]
        woTc = np.ascontiguousarray(
            Wo[:, g * 256 : (g + 1) * 256].T, dtype=np.float32
        )                                             # [256, C]
        in_maps.append(
            {
                "xT": xTc,
                "wr": np.ascontiguousarray(wrc),
                "cosd": cosf,
                "sind": sinf,
                "ve3": ve3,
                "woT": woTc,
                "tri": tri,
            }
        )
    return in_maps


_PROGRAM = None


def kernel(x, ve, cos, sin, Wq, Wk, Wv, Wo, Wg, _trace=False):
    from concourse.bass_utils import run_bass_kernel_spmd

    # coerce to host fp32 ndarrays up front (harness may pass jax arrays)
    x, ve, cos, sin, Wq, Wk, Wv, Wo, Wg = (
        np.asarray(a, dtype=np.float32)
        for a in (x, ve, cos, sin, Wq, Wk, Wv, Wo, Wg)
    )
    global _PROGRAM
    if _PROGRAM is None:
        _PROGRAM = build_program()
    nc = _PROGRAM
    in_maps = make_core_inputs(x, ve, cos, sin, Wq, Wk, Wv, Wo, Wg)
    res = run_bass_kernel_spmd(nc, in_maps, list(range(8)), trace=_trace)
    outs = [r["out"] for r in res.results]
    full = np.zeros((B, T, C), np.float32)
    for c in range(8):
        full[c // N_KV_HEAD] += outs[c]
    if _trace:
        kernel.last_results = res
    return full

